# revision 1
# baseline (speedup 1.0000x reference)
"""Causal self-attention (RoPE) Trainium2 Bass kernel.

Sharding: 8 cores = 4 batches x 2 head-groups. Core c handles batch c//2 and
heads (c%2)*8 .. (c%2)*8+7. Each core computes its QKV projection slice, RoPE,
causal flash-style attention in transposed layout, and a partial output
projection; the host sums the two partial projections per batch.

All matmuls use float32r (TF32-like, ~1.5e-4 rel err) at full PE rate.
Attention is computed transposed (s^T = k q^T) so softmax denominators come
from an appended ones-column in the value matrix and attention output feeds
the output projection as lhsT with no transposes.
"""

import math
import numpy as np
from contextlib import ExitStack

import concourse.bass as bass
import concourse.tile as tile
from concourse import bacc, mybir
from concourse.bass_utils import run_bass_kernel_spmd

F32 = mybir.dt.float32
R32 = mybir.dt.float32r
EXPF = mybir.ActivationFunctionType.Exp
MULT = mybir.AluOpType.mult
ADD = mybir.AluOpType.add

B, T, C, H, D = 4, 2048, 1024, 16, 64
HL = 8            # local heads per core
NP = HL // 2      # head pairs per core
KT = C // 128     # contraction tiles for projections
TT = T // 128     # 128-row tiles of T
QC = T // 512     # 512-col chunks of T
SCALE = 1.0 / math.sqrt(D)

_CACHE = {}


def _build_nc():
    nc = bacc.Bacc("TRN2", debug=False, num_devices=8)

    xT_d = nc.dram_tensor("xT", [KT, 128, T], R32, kind="ExternalInput").ap()
    wq_d = nc.dram_tensor("wq", [128, NP, KT, 128], R32, kind="ExternalInput").ap()
    wk_d = nc.dram_tensor("wk", [128, NP, KT, 128], R32, kind="ExternalInput").ap()
    wv_d = nc.dram_tensor("wv", [128, KT, 512], R32, kind="ExternalInput").ap()
    wo_d = nc.dram_tensor("wo", [128, NP, C], R32, kind="ExternalInput").ap()
    cos_d = nc.dram_tensor("cosT", [128, T], F32, kind="ExternalInput").ap()
    sin_d = nc.dram_tensor("sinT", [128, T], F32, kind="ExternalInput").ap()
    psw_d = nc.dram_tensor("psw", [128, 128], R32, kind="ExternalInput").ap()
    e64_d = nc.dram_tensor("e64", [128, 64], R32, kind="ExternalInput").ap()
    msk_d = nc.dram_tensor("msk", [128, 4, 512], F32, kind="ExternalInput").ap()
    wrm_d = nc.dram_tensor("wrm", [128, 512], R32, kind="ExternalInput").ap()
    out_d = nc.dram_tensor("out", [T, C], F32, kind="ExternalOutput").ap()

    with tile.TileContext(nc) as tc:
        with ExitStack() as ctx:
            pers = ctx.enter_context(tc.tile_pool(name="pers", bufs=1))
            vext = pers.tile([128, TT, HL, D + 1], R32)
            qkT = {}
            for p in range(NP):
                for s in "qk":
                    qkT[(p, s)] = pers.tile([128, T], R32, name=f"qkT_{p}_{s}")
            yT = [pers.tile([128, T], R32, name=f"yT_{r}") for r in range(NP)]
            nc.gpsimd.memset(vext[:, :, :, D].bitcast(F32), 1.0)

            # ---- phase V: value projection -> vext (natural layout + ones col)
            with (
                tc.tile_pool(name="vph", bufs=2) as vp,
                tc.tile_pool(name="vw", bufs=1) as vw,
                tc.tile_pool(name="vps", bufs=2, space="PSUM") as vps,
            ):
                wv_sb = vw.tile([128, KT, 512], R32)
                nc.sync.dma_start(wv_sb[:], wv_d)
                for tt in range(TT):
                    xv = vp.tile([128, KT, 128], R32, tag="xv")
                    nc.sync.dma_start(
                        xv[:],
                        xT_d[:, :, tt * 128 : (tt + 1) * 128].rearrange(
                            "k c t -> c k t"
                        ),
                    )
                    ps = vps.tile([128, 512], F32, tag="pv")
                    for kt in range(KT):
                        nc.tensor.matmul(
                            ps[:], xv[:, kt], wv_sb[:, kt],
                            start=(kt == 0), stop=(kt == KT - 1),
                        )
                    nc.vector.tensor_copy(vext[:, tt, :, 0:D], ps[:])

            # ---- phase QK: q/k projection + RoPE -> qkT (transposed layout)
            with (
                tc.tile_pool(name="qkst", bufs=2) as sp,
                tc.tile_pool(name="qkw", bufs=1) as qw,
                tc.tile_pool(name="ctab", bufs=2) as ct,
                tc.tile_pool(name="qkps", bufs=2, space="PSUM") as qps,
                tc.tile_pool(name="rotps", bufs=2, space="PSUM") as rps,
            ):
                wq_sb = qw.tile([128, NP, KT, 128], R32)
                wk_sb = qw.tile([128, NP, KT, 128], R32)
                psw_sb = qw.tile([128, 128], R32)
                nc.sync.dma_start(wq_sb[:], wq_d)
                nc.sync.dma_start(wk_sb[:], wk_d)
                nc.sync.dma_start(psw_sb[:], psw_d)
                for qc in range(QC):
                    lo, hi = qc * 512, (qc + 1) * 512
                    xc = sp.tile([128, KT, 512], R32, tag="xc")
                    nc.sync.dma_start(
                        xc[:], xT_d[:, :, lo:hi].rearrange("k c t -> c k t")
                    )
                    cosc = ct.tile([128, 512], F32, tag="cosc")
                    sinc = ct.tile([128, 512], F32, tag="sinc")
                    nc.sync.dma_start(cosc[:], cos_d[:, lo:hi])
                    nc.sync.dma_start(sinc[:], sin_d[:, lo:hi])
                    for p in range(NP):
                        for w_sb, key in ((wq_sb, "q"), (wk_sb, "k")):
                            dst = qkT[(p, key)][:, lo:hi]
                            ps = qps.tile([128, 512], F32, tag="pq")
                            for kt in range(KT):
                                nc.tensor.matmul(
                                    ps[:], w_sb[:, p, kt], xc[:, kt],
                                    start=(kt == 0), stop=(kt == KT - 1),
                                )
                            nc.vector.tensor_tensor(
                                dst, ps[:], cosc[:], MULT
                            )
                            u = sp.tile([128, 512], R32, tag="u")
                            nc.vector.tensor_tensor(
                                u[:], ps[:], sinc[:], MULT
                            )
                            pr = rps.tile([128, 512], F32, tag="pr")
                            nc.tensor.matmul(
                                pr[:], psw_sb[:], u[:], start=True, stop=True
                            )
                            nc.vector.tensor_tensor(dst, pr[:], dst, ADD)

            # ---- phase ATT: causal attention per head pair, transposed
            with (
                tc.tile_pool(name="attp", bufs=3) as ap_,
                tc.tile_pool(name="atab", bufs=1) as at_,
                tc.tile_pool(name="sps", bufs=2, space="PSUM") as sps,
                tc.tile_pool(name="yps", bufs=2, space="PSUM") as yps,
            ):
                e64_sb = at_.tile([128, 64], R32)
                msk_sb = at_.tile([128, 4, 512], F32)
                rrec = at_.tile([128, 512], R32)
                rscr = at_.tile([128, 512], F32)
                rscr2 = at_.tile([128, 512], F32)
                ww_sb = at_.tile([128, 128], R32)
                wrm_sb = at_.tile([128, 512], R32)
                nc.sync.dma_start(e64_sb[:], e64_d)
                nc.sync.dma_start(msk_sb[:], msk_d)
                nc.sync.dma_start(ww_sb[:], psw_d)
                nc.sync.dma_start(wrm_sb[:], wrm_d)
                nc.gpsimd.memset(rrec[:].bitcast(F32), 0.0)
                for p in range(NP):
                    qTt = qkT[(p, "q")]
                    kTt = qkT[(p, "k")]
                    for qc in range(QC):
                        lo, hi = qc * 512, (qc + 1) * 512
                        nkt = (qc + 1) * 4
                        psyA = yps.tile([65, 512], F32, tag="yA")
                        psyB = yps.tile([65, 512], F32, tag="yB")
                        for kt in range(nkt):
                            first, last = kt == 0, kt == nkt - 1
                            klo, khi = kt * 128, (kt + 1) * 128
                            off = klo - lo
                            # valid q range for this tile is [off, 512); trim
                            # to it when the fp32r fast path allows (>=256)
                            tr = off if off in (128, 256) else 0
                            w = 512 - tr
                            ps2 = sps.tile([128, 1024], F32, tag="sA")
                            p3 = ps2[:].rearrange("p (h n) -> p h n", h=2)
                            nc.tensor.matmul(
                                ps2[:, tr:512],
                                kTt[0:64, klo:khi], qTt[0:64, lo + tr : hi],
                                start=True, stop=True,
                            )
                            nc.tensor.matmul(
                                ps2[:, 512 + tr : 1024],
                                kTt[64:128, klo:khi], qTt[64:128, lo + tr : hi],
                                start=True, stop=True,
                            )
                            aAB = ap_.tile([128, 1024], R32, tag="aA")
                            a3 = aAB[:].rearrange("p (h n) -> p h n", h=2)
                            aA = aAB[:, 0:512]
                            aB = aAB[:, 512:1024]
                            nc.scalar.activation(
                                a3[:, :, tr:512], p3[:, :, tr:512],
                                EXPF, scale=SCALE,
                            )
                            if off >= 0:
                                mi = off // 128
                                nc.vector.tensor_tensor(
                                    aA[:, tr:512], aA[:, tr:512],
                                    msk_sb[:, mi, tr:512], MULT,
                                )
                                nc.vector.tensor_tensor(
                                    aB[:, tr:512], aB[:, tr:512],
                                    msk_sb[:, mi, tr:512], MULT,
                                )
                            nc.tensor.matmul(
                                psyA[:, tr:512], vext[:, kt, 2 * p, :],
                                aA[:, tr:512], start=first, stop=last,
                            )
                            nc.tensor.matmul(
                                psyB[:, tr:512], vext[:, kt, 2 * p + 1, :],
                                aB[:, tr:512], start=first, stop=last,
                            )
                        # normalize: recip of denom row, broadcast via selector
                        # matmul, multiply into yT
                        for hh, psy in ((0, psyA), (1, psyB)):
                            with nc.allow_low_precision(
                                reason="recip row feeds fp32r selector matmul"
                            ):
                                nc.vector.reciprocal(
                                    rrec[64:65, :], psy[64:65, :]
                                )
                            pbc = sps.tile([64, 512], F32, tag="sA", name="pbc")
                            nc.tensor.matmul(
                                pbc[:], e64_sb[:], rrec[:], start=True, stop=True
                            )
                            bc = ap_.tile([64, 512], R32, tag="bc")
                            nc.scalar.copy(bc[:], pbc[:])
                            if hh == 0:
                                nc.vector.tensor_tensor(
                                    yT[p][0:64, lo:hi], psy[0:64, :], bc[:], MULT
                                )
                            else:
                                tb = ap_.tile([64, 512], R32, tag="tb")
                                nc.vector.tensor_tensor(
                                    tb[:], psy[0:64, :], bc[:], MULT
                                )
                                nc.sync.dma_start(yT[p][64:128, lo:hi], tb[:])

            # ---- phase OUT: output projection (partial; host sums over cores)
            with (
                tc.tile_pool(name="oph", bufs=3) as op_,
                tc.tile_pool(name="ow", bufs=1) as ow,
                tc.tile_pool(name="ops", bufs=4, space="PSUM") as ops,
            ):
                wo_sb = ow.tile([128, NP, C], R32)
                nc.sync.dma_start(wo_sb[:], wo_d)
                for mt in range(TT):
                    mlo, mhi = mt * 128, (mt + 1) * 128
                    for cc in range(2):
                        clo, chi = cc * 512, (cc + 1) * 512
                        ps = ops.tile([128, 512], F32, tag="po")
                        for r in range(NP):
                            nc.tensor.matmul(
                                ps[:], yT[r][:, mlo:mhi], wo_sb[:, r, clo:chi],
                                start=(r == 0), stop=(r == NP - 1),
                            )
                        ob = op_.tile([128, 512], F32, tag="ob")
                        nc.vector.tensor_copy(ob[:], ps[:])
                        nc.sync.dma_start(out_d[mlo:mhi, clo:chi], ob[:])

    nc.compile()
    return nc


def _host_tables():
    half = D // 2
    freq = np.exp(-math.log(10000.0) * np.arange(half) / half).astype(np.float64)
    ang = np.arange(T, dtype=np.float64)[None, :] * freq[:, None]  # [32, T]
    cos32 = np.cos(ang).astype(np.float32)
    sin32 = np.sin(ang).astype(np.float32)
    cosT = np.tile(cos32, (4, 1))                                   # [128, T]
    sinT = np.concatenate([sin32, -sin32, sin32, -sin32], axis=0)   # [128, T]
    psw = np.zeros((128, 128), np.float32)
    psw[np.arange(128) ^ 32, np.arange(128)] = 1.0
    e64 = np.zeros((128, 64), np.float32)
    e64[64, :] = 1.0
    kk = np.arange(128)[:, None, None]
    ii = np.arange(4)[None, :, None]
    qq = np.arange(512)[None, None, :]
    msk = (qq >= kk + ii * 128).astype(np.float32)
    return cosT, sinT, psw, e64, msk


def _pack_weights(w_qkv, w_out, hg):
    lo, hi = hg * HL, (hg + 1) * HL
    wqf = w_qkv[:, 0:C].reshape(C, H, D)[:, lo:hi]       # [C, 8, D]
    wkf = w_qkv[:, C : 2 * C].reshape(C, H, D)[:, lo:hi]
    wvf = w_qkv[:, 2 * C : 3 * C].reshape(C, H, D)[:, lo:hi]

    def pack_qk(w):
        a = w.reshape(KT, 128, NP, 2, D)
        return np.ascontiguousarray(
            a.transpose(1, 2, 0, 3, 4).reshape(128, NP, KT, 128)
        )

    wq = pack_qk(wqf)
    wk = pack_qk(wkf)
    wv = np.ascontiguousarray(
        wvf.reshape(KT, 128, HL * D).transpose(1, 0, 2)
    )
    wo_l = w_out.reshape(H, D, C)[lo:hi].reshape(NP, 128, C)
    wo = np.ascontiguousarray(wo_l.transpose(1, 0, 2))
    return wq, wk, wv, wo


def kernel(x, w_qkv, w_out):
    x = np.asarray(x, dtype=np.float32)
    w_qkv = np.asarray(w_qkv, dtype=np.float32)
    w_out = np.asarray(w_out, dtype=np.float32)

    if "nc" not in _CACHE:
        _CACHE["nc"] = _build_nc()
    nc = _CACHE["nc"]

    cosT, sinT, psw, e64, msk = _host_tables()
    packs = [_pack_weights(w_qkv, w_out, hg) for hg in range(2)]
    xTs = [
        np.ascontiguousarray(x[b].T).reshape(KT, 128, T) for b in range(B)
    ]

    in_maps = []
    for c in range(8):
        b, hg = c // 2, c % 2
        wq, wk, wv, wo = packs[hg]
        in_maps.append(
            {
                "xT": xTs[b], "wq": wq, "wk": wk, "wv": wv, "wo": wo,
                "cosT": cosT, "sinT": sinT, "psw": psw, "e64": e64,
                "msk": msk, "wrm": np.full((128, 512), 0.03, np.float32),
            }
        )

    res = run_bass_kernel_spmd(nc, in_maps, core_ids=list(range(8)))
    outs = [res.results[c]["out"] for c in range(8)]
    y = np.stack([outs[2 * b] + outs[2 * b + 1] for b in range(B)], axis=0)
    return y.astype(np.float32)



# revision 4
# speedup vs baseline: 1.7978x; 1.7978x over previous
"""Causal self-attention (RoPE) Trainium2 Bass kernel, v2 (bf16).

Sharding: 8 cores = 4 batches x 2 head-groups. Core c handles batch c//2 and
heads (c%2)*8 .. (c%2)*8+7. Each core computes its QKV projection slice, RoPE,
causal flash-style attention in transposed layout, and a partial output
projection; the host sums the two partial projections per batch.

v2 changes vs baseline:
- all matmuls bf16 (fp32r ran in fp32-HIGH mode at half PE rate and kept the
  HAM clock gate cold for ~2/3 of the kernel)
- attention inner loop software-pipelined: scores for tile k+1 issue on PE
  before the AV matmuls of tile k, so exp (ACT) and mask (DVE) overlap PE
- reciprocal_approx_fast for softmax denominators (plain RECIPROCAL was 3.3us
  per call, 107us total)
- causal mask multiply only on the 128-wide diagonal triangle
- single pass over x feeds both the V and QK projections
- projection / attention / output-projection emission interleaved per
  512-query chunk so the PE queue always has ready work behind stalled ops
"""

import math
import numpy as np
from contextlib import ExitStack

import ml_dtypes

import concourse.bass as bass
import concourse.tile as tile
from concourse import bacc, mybir
from concourse.bass_utils import run_bass_kernel_spmd

F32 = mybir.dt.float32
BF16 = mybir.dt.bfloat16
EXPF = mybir.ActivationFunctionType.Exp
MULT = mybir.AluOpType.mult
ADD = mybir.AluOpType.add

B, T, C, H, D = 4, 2048, 1024, 16, 64
HL = 8            # local heads per core
NP = HL // 2      # head pairs per core
KT = C // 128     # contraction tiles for projections
TT = T // 128     # 128-row tiles of T
QC = T // 512     # 512-col chunks of T
SCALE = 1.0 / math.sqrt(D)

_CACHE = {}


def _build_nc():
    nc = bacc.Bacc("TRN2", debug=False, num_devices=8)

    xT_d = nc.dram_tensor("xT", [128, KT, T], BF16, kind="ExternalInput").ap()
    wq_d = nc.dram_tensor("wq", [128, NP, KT, 128], BF16, kind="ExternalInput").ap()
    wk_d = nc.dram_tensor("wk", [128, NP, KT, 128], BF16, kind="ExternalInput").ap()
    wv_d = nc.dram_tensor("wv", [128, KT, 512], BF16, kind="ExternalInput").ap()
    wo_d = nc.dram_tensor("wo", [128, NP, C], BF16, kind="ExternalInput").ap()
    cos_d = nc.dram_tensor("cosT", [128, T], BF16, kind="ExternalInput").ap()
    sin_d = nc.dram_tensor("sinT", [128, T], BF16, kind="ExternalInput").ap()
    psw_d = nc.dram_tensor("psw", [128, 128], BF16, kind="ExternalInput").ap()
    e64_d = nc.dram_tensor("e64", [128, 64], BF16, kind="ExternalInput").ap()
    tri_d = nc.dram_tensor("tri", [128, 128], BF16, kind="ExternalInput").ap()
    out_d = nc.dram_tensor("out", [T, C], BF16, kind="ExternalOutput").ap()

    with tile.TileContext(nc) as tc:
        with ExitStack() as ctx:
            pers = ctx.enter_context(tc.tile_pool(name="pers", bufs=1))
            wts = ctx.enter_context(tc.tile_pool(name="wts", bufs=1))
            xcp = ctx.enter_context(tc.tile_pool(name="xcp", bufs=2))
            wrk = ctx.enter_context(tc.tile_pool(name="wrk", bufs=3))
            rwk = ctx.enter_context(tc.tile_pool(name="rwk", bufs=2))
            ps_sc = ctx.enter_context(
                tc.tile_pool(name="ps_sc", bufs=2, space="PSUM")
            )
            ps_py = ctx.enter_context(
                tc.tile_pool(name="ps_py", bufs=2, space="PSUM")
            )

            # ---- persistent tensors
            qkT = {}
            for p in range(NP):
                for s in "qk":
                    qkT[(p, s)] = pers.tile([128, T], BF16, name=f"qkT_{p}_{s}")
            vext = pers.tile([128, TT, HL, 66], BF16)
            yT = [pers.tile([128, T], BF16, name=f"yT_{r}") for r in range(NP)]
            rrec = [pers.tile([128, 1024], F32, name=f"rrec{i}") for i in range(2)]
            rrecb = [pers.tile([128, 1024], BF16, name=f"rrecb{i}") for i in range(2)]

            nc.gpsimd.memset(vext[:, :, :, 64:66], 1.0)
            for i in range(2):
                nc.gpsimd.memset(rrec[i][:], 0.0)
                nc.gpsimd.memset(rrecb[i][:], 0.0)

            # ---- weights / tables
            wq_sb = wts.tile([128, NP, KT, 128], BF16)
            wk_sb = wts.tile([128, NP, KT, 128], BF16)
            wv_sb = wts.tile([128, KT, 512], BF16)
            wo_sb = wts.tile([128, NP, C], BF16)
            cos_sb = wts.tile([128, T], BF16)
            sin_sb = wts.tile([128, T], BF16)
            psw_sb = wts.tile([128, 128], BF16)
            e64_sb = wts.tile([128, 64], BF16)
            tri_sb = wts.tile([128, 128], BF16)
            nc.sync.dma_start(wq_sb[:], wq_d)
            nc.sync.dma_start(wk_sb[:], wk_d)
            nc.sync.dma_start(wv_sb[:], wv_d)
            nc.sync.dma_start(wo_sb[:], wo_d)
            nc.sync.dma_start(cos_sb[:], cos_d)
            nc.sync.dma_start(sin_sb[:], sin_d)
            nc.sync.dma_start(psw_sb[:], psw_d)
            nc.sync.dma_start(e64_sb[:], e64_d)
            nc.sync.dma_start(tri_sb[:], tri_d)

            xc = {}

            def load_xc(qc):
                xc[qc] = xcp.tile([128, KT, 512], BF16, tag="xc", name=f"xc{qc}")
                nc.sync.dma_start(
                    xc[qc][:], xT_d[:, :, qc * 512 : (qc + 1) * 512]
                )

            def vproj_tile(tt):
                # one 128-row tile of the value projection -> vext
                qc = tt // 4
                toff = (tt % 4) * 128
                ps = ps_sc.tile([128, 1024], F32, tag="sc", name="vps")
                for kt in range(KT):
                    nc.tensor.matmul(
                        ps[:, 0:512],
                        xc[qc][:, kt, toff : toff + 128],
                        wv_sb[:, kt],
                        start=(kt == 0),
                        stop=(kt == KT - 1),
                    )
                nc.vector.tensor_copy(
                    vext[:, tt, :, 0:64],
                    ps[:, 0:512].rearrange("p (h d) -> p h d", h=HL),
                )

            def qkproj_mms(p, qc):
                # q into cols 0:512, k into cols 512:1024 of one py-tag tile
                ps = ps_py.tile([128, 1024], F32, tag="py", name=f"qk_{p}_{qc}")
                for kt in range(KT):
                    nc.tensor.matmul(
                        ps[:, 0:512], wq_sb[:, p, kt], xc[qc][:, kt],
                        start=(kt == 0), stop=(kt == KT - 1),
                    )
                for kt in range(KT):
                    nc.tensor.matmul(
                        ps[:, 512:1024], wk_sb[:, p, kt], xc[qc][:, kt],
                        start=(kt == 0), stop=(kt == KT - 1),
                    )
                return ps

            def rope(p, qc, ps):
                # qkT[.] = cos*proj + psw_perm(sin*proj), per q/k half of ps
                lo, hi = qc * 512, (qc + 1) * 512
                qsb = rwk.tile([128, 1024], BF16, tag="qsb")
                nc.vector.tensor_copy(qsb[:], ps[:])
                u = rwk.tile([128, 1024], BF16, tag="u")
                qcs = rwk.tile([128, 1024], BF16, tag="qcs")
                for h in range(2):
                    sl = slice(h * 512, h * 512 + 512)
                    nc.vector.tensor_tensor(
                        u[:, sl], qsb[:, sl], sin_sb[:, lo:hi], MULT
                    )
                    nc.vector.tensor_tensor(
                        qcs[:, sl], qsb[:, sl], cos_sb[:, lo:hi], MULT
                    )
                # rotation matmuls overwrite ps (proj value already in qsb)
                for h in range(2):
                    sl = slice(h * 512, h * 512 + 512)
                    nc.tensor.matmul(
                        ps[:, sl], psw_sb[:], u[:, sl], start=True, stop=True
                    )
                for h, s in ((0, "q"), (1, "k")):
                    sl = slice(h * 512, h * 512 + 512)
                    nc.vector.tensor_tensor(
                        qkT[(p, s)][:, lo:hi], ps[:, sl], qcs[:, sl], ADD
                    )

            def outproj_piece(qc, piece):
                # one of 8 pieces: 128 query rows x 512 out cols
                mt = qc * 4 + piece // 2
                cc = piece % 2
                mlo, mhi = mt * 128, (mt + 1) * 128
                clo, chi = cc * 512, (cc + 1) * 512
                ps = ps_sc.tile([128, 1024], F32, tag="sc", name="ops")
                for r in range(NP):
                    nc.tensor.matmul(
                        ps[:, 0:512], yT[r][:, mlo:mhi], wo_sb[:, r, clo:chi],
                        start=(r == 0), stop=(r == NP - 1),
                    )
                ob = wrk.tile([128, 512], BF16, tag="ob")
                nc.vector.tensor_copy(ob[:], ps[:, 0:512])
                nc.sync.dma_start(out_d[mlo:mhi, clo:chi], ob[:])

            def att_stream(p, qc):
                qT = qkT[(p, "q")]
                kT = qkT[(p, "k")]
                lo, hi = qc * 512, (qc + 1) * 512
                nkt = (qc + 1) * 4
                psy = ps_py.tile([128, 1024], F32, tag="py", name=f"psy_{p}_{qc}")

                tiles = []  # (kt, tr, sc_tile, a_tile)

                def emit_scores(kt):
                    klo, khi = kt * 128, (kt + 1) * 128
                    off = klo - lo
                    tr = off if off > 0 else 0
                    st = ps_sc.tile([128, 1024], F32, tag="sc", name="st")
                    nc.tensor.matmul(
                        st[:, tr:512],
                        kT[0:64, klo:khi], qT[0:64, lo + tr : hi],
                        start=True, stop=True,
                    )
                    nc.tensor.matmul(
                        st[:, 512 + tr : 1024],
                        kT[64:128, klo:khi], qT[64:128, lo + tr : hi],
                        start=True, stop=True,
                    )
                    a = wrk.tile([128, 1024], BF16, tag="a", name="a")
                    s3 = st[:].rearrange("p (h n) -> p h n", h=2)
                    a3 = a[:].rearrange("p (h n) -> p h n", h=2)
                    nc.scalar.activation(
                        a3[:, :, tr:512], s3[:, :, tr:512], EXPF, scale=SCALE
                    )
                    if off >= 0:
                        for h in range(2):
                            nc.vector.tensor_tensor(
                                a3[:, h, off : off + 128],
                                a3[:, h, off : off + 128],
                                tri_sb[:],
                                MULT,
                            )
                    tiles.append((kt, tr, st, a))

                def emit_av(i):
                    kt, tr, st, a = tiles[i]
                    first, last = kt == 0, kt == nkt - 1
                    for h in range(2):
                        nc.tensor.matmul(
                            psy[0:65, h * 512 + tr : h * 512 + 512],
                            vext[:, kt, 2 * p + h, 0:65],
                            a[:, h * 512 + tr : h * 512 + 512],
                            start=first, stop=last,
                        )

                emit_scores(0)
                for kt in range(1, nkt):
                    emit_scores(kt)
                    emit_av(kt - 1)
                emit_av(nkt - 1)

                # denominator reciprocal (row 64 = ones-column output).
                # NOTE: custom-DVE ops misplace partitions on HW when the AP
                # base partition is nonzero, so run over all 128 partitions
                # (rows != 64 are garbage, discarded by the row-64-only cast).
                rr, rb = rrec[p % 2], rrecb[p % 2]
                nc.vector.reciprocal_approx_fast(rr[:], psy[:])
                nc.vector.tensor_copy(rb[64:65, :], rr[64:65, :])
                return psy

            def finish_normalize(p, qc, psy):
                lo, hi = qc * 512, (qc + 1) * 512
                rb = rrecb[p % 2]
                pbc = ps_sc.tile([128, 1024], F32, tag="sc", name="pbc")
                for h in range(2):
                    sl = slice(h * 512, h * 512 + 512)
                    nc.tensor.matmul(
                        pbc[0:64, sl], e64_sb[:], rb[:, sl],
                        start=True, stop=True,
                    )
                bcs = wrk.tile([64, 1024], BF16, tag="bcs")
                nc.scalar.copy(bcs[:], pbc[0:64, :])
                nc.vector.tensor_tensor(
                    yT[p][0:64, lo:hi], psy[0:64, 0:512], bcs[:, 0:512], MULT
                )
                tb = wrk.tile([64, 512], BF16, tag="tb")
                nc.vector.tensor_tensor(
                    tb[:], psy[0:64, 512:1024], bcs[:, 512:1024], MULT
                )
                nc.sync.dma_start(yT[p][64:128, lo:hi], tb[:])

            # ---- emission ------------------------------------------------
            load_xc(0)
            load_xc(1)
            for tt in range(4):
                vproj_tile(tt)
            for p in range(NP):
                ps = qkproj_mms(p, 0)
                rope(p, 0, ps)

            for qc in range(QC):
                if qc + 2 < QC:
                    load_xc(qc + 2)
                for p in range(NP):
                    psy = att_stream(p, qc)
                    if qc < QC - 1:
                        ps = qkproj_mms(p, qc + 1)
                    if qc < QC - 1 and p in (1, 2):
                        base = (qc + 1) * 4
                        for tt in (base + 2 * (p - 1), base + 2 * (p - 1) + 1):
                            vproj_tile(tt)
                    if qc > 0:
                        outproj_piece(qc - 1, 2 * p)
                        outproj_piece(qc - 1, 2 * p + 1)
                    if qc < QC - 1:
                        rope(p, qc + 1, ps)
                    finish_normalize(p, qc, psy)
            for piece in range(8):
                outproj_piece(QC - 1, piece)

    nc.compile()
    return nc


def _host_tables():
    half = D // 2
    freq = np.exp(-math.log(10000.0) * np.arange(half) / half).astype(np.float64)
    ang = np.arange(T, dtype=np.float64)[None, :] * freq[:, None]  # [32, T]
    cos32 = np.cos(ang).astype(np.float32)
    sin32 = np.sin(ang).astype(np.float32)
    cosT = np.tile(cos32, (4, 1))                                   # [128, T]
    sinT = np.concatenate([sin32, -sin32, sin32, -sin32], axis=0)   # [128, T]
    psw = np.zeros((128, 128), np.float32)
    psw[np.arange(128) ^ 32, np.arange(128)] = 1.0
    e64 = np.zeros((128, 64), np.float32)
    e64[64, :] = 1.0
    kk = np.arange(128)[:, None]
    qq = np.arange(128)[None, :]
    tri = (qq >= kk).astype(np.float32)
    return cosT, sinT, psw, e64, tri


def _bf(a):
    return np.ascontiguousarray(a.astype(ml_dtypes.bfloat16))


def _pack_weights(w_qkv, w_out, hg):
    lo, hi = hg * HL, (hg + 1) * HL
    wqf = w_qkv[:, 0:C].reshape(C, H, D)[:, lo:hi]       # [C, 8, D]
    wkf = w_qkv[:, C : 2 * C].reshape(C, H, D)[:, lo:hi]
    wvf = w_qkv[:, 2 * C : 3 * C].reshape(C, H, D)[:, lo:hi]

    def pack_qk(w):
        a = w.reshape(KT, 128, NP, 2, D)
        return _bf(a.transpose(1, 2, 0, 3, 4).reshape(128, NP, KT, 128))

    wq = pack_qk(wqf)
    wk = pack_qk(wkf)
    wv = _bf(wvf.reshape(KT, 128, HL * D).transpose(1, 0, 2))
    wo_l = w_out.reshape(H, D, C)[lo:hi].reshape(NP, 128, C)
    wo = _bf(wo_l.transpose(1, 0, 2))
    return wq, wk, wv, wo


def _prepare_in_maps(x, w_qkv, w_out):
    x = np.asarray(x, dtype=np.float32)
    w_qkv = np.asarray(w_qkv, dtype=np.float32)
    w_out = np.asarray(w_out, dtype=np.float32)

    cosT, sinT, psw, e64, tri = _host_tables()
    cosT, sinT, psw, e64, tri = map(_bf, (cosT, sinT, psw, e64, tri))
    packs = [_pack_weights(w_qkv, w_out, hg) for hg in range(2)]
    xTs = [
        _bf(x[b].T.reshape(KT, 128, T).transpose(1, 0, 2)) for b in range(B)
    ]

    in_maps = []
    for c in range(8):
        b, hg = c // 2, c % 2
        wq, wk, wv, wo = packs[hg]
        in_maps.append(
            {
                "xT": xTs[b], "wq": wq, "wk": wk, "wv": wv, "wo": wo,
                "cosT": cosT, "sinT": sinT, "psw": psw, "e64": e64,
                "tri": tri,
            }
        )
    return in_maps


def kernel(x, w_qkv, w_out):
    if "nc" not in _CACHE:
        _CACHE["nc"] = _build_nc()
    nc = _CACHE["nc"]

    in_maps = _prepare_in_maps(x, w_qkv, w_out)
    res = run_bass_kernel_spmd(nc, in_maps, core_ids=list(range(8)))
    outs = [res.results[c]["out"].astype(np.float32) for c in range(8)]
    y = np.stack([outs[2 * b] + outs[2 * b + 1] for b in range(B)], axis=0)
    return y.astype(np.float32)
